# revision 21
# baseline (speedup 1.0000x reference)
"""Trainium2 Bass kernel for nn_GeneralizedAttention (Performer-style linear
attention with GELU random features).

Math (per (b,h)):
    qp  = gelu(q @ proj^T)            [n, m]
    kp  = gelu(k @ proj^T)            [n, m]
    ksum= kp.sum(n)                   [m]   (v ones column)
    ctx = kp^T @ v                    [m, e]
    den = qp @ ksum                   [n]   (ctx_aug col 64)
    out = (qp @ ctx) / den[:, None]   [n, e]

Sharding: B*H = 64 (b,h) pairs split across 8 cores, 8 pairs each; proj_mat
replicated; no cross-core comms.

Layout: n is chunked as n = p*32 + c (p = SBUF partition, c = chunk id, 32
chunks of 128); all HBM<->SBUF transfers are contiguous in (c, d) per
partition (4-8KB descriptor runs -> full DMA bus efficiency). Chunks pair
(2u, 2u+1) onto partition halves (t*64+d) of qT/kT so projection matmuls
contract d=64 on both row halves. The ctx and out contractions stream the
narrow e+1=65 axis with the feature tiles as stationary weights (half the
PE time of streaming m), and produce `out` directly in [n, e] — no output
transposes. Accumulation groups never interleave within a PSUM bank (a
group start clears has_written for the whole bank), and every PSUM
accumulator tile is padded to a full 2KB bank.
"""

import numpy as np

B, H, N, D, M = 4, 16, 4096, 64, 256
NCORES = 8
BH = B * H
BHPC = BH // NCORES  # 8 (b,h) pairs per core
P = 128
NCH = N // P         # 32 chunks of 128 n
NU = NCH // 2        # 16 chunk pairs
EAUG = D + 1         # 65: e plus the folded ksum/den column


def _emit_body(ctx, tc, out_d, q_d, k_d, v_d, proj_d, bhpc, repeat=1):
    import concourse.bass as bass
    import concourse.mybir as mybir
    from concourse.masks import make_identity

    nc = tc.nc
    f32 = mybir.dt.float32
    bf16 = mybir.dt.bfloat16
    MULT = mybir.AluOpType.mult
    GELU = mybir.ActivationFunctionType.Gelu

    const = ctx.enter_context(tc.tile_pool(name="const", bufs=1))
    inp = ctx.enter_context(tc.tile_pool(name="inp", bufs=6))
    vpool = ctx.enter_context(tc.tile_pool(name="vpool", bufs=3))
    tsb = ctx.enter_context(tc.tile_pool(name="tsb", bufs=6))
    feat = ctx.enter_context(tc.tile_pool(name="feat", bufs=3))
    small = ctx.enter_context(tc.tile_pool(name="small", bufs=3))
    outp = ctx.enter_context(tc.tile_pool(name="outp", bufs=3))
    ps_gen = ctx.enter_context(tc.tile_pool(name="ps_gen", bufs=2, space="PSUM"))
    ps_small = ctx.enter_context(tc.tile_pool(name="ps_small", bufs=2, space="PSUM"))
    ps_acc = ctx.enter_context(tc.tile_pool(name="ps_acc", bufs=2, space="PSUM"))

    ident_bf = const.tile([P, P], bf16, name="ident_bf")
    make_identity(nc, ident_bf)
    ident_f32 = const.tile([P, P], f32, name="ident_f32")
    make_identity(nc, ident_f32)

    # proj^T [d, m] duplicated on both partition halves (rows 0-63 and 64-127)
    proj_nat = const.tile([P, 2, D], f32, name="proj_nat")
    nc.sync.dma_start(proj_nat[:], proj_d.rearrange("(t p) d -> p t d", p=P))
    projT = const.tile([P, M], bf16, name="projT")
    for t in range(2):
        pspt = ps_small.tile([D, P], f32, tag="small", name=f"ps_projT{t}")
        nc.tensor.transpose(pspt[:], proj_nat[:, t, :], ident_f32)
        nc.vector.tensor_copy(projT[0:D, P * t : P * (t + 1)], pspt[:])
        nc.vector.tensor_copy(projT[D:P, P * t : P * (t + 1)], pspt[:])

    unroll = 1
    if repeat > 1:
        # amortize the For_i semaphore-reset barrier over several passes
        for cand in (16, 8, 4, 2):
            if repeat % cand == 0:
                unroll = cand
                break
        loop_cm = tc.For_i(0, repeat // unroll, 1)
        loop_cm.__enter__()

    for bh in range(bhpc * unroll):
        bh = bh % bhpc
        # ---- loads (SWDGE cast f32 -> bf16), contiguous (c,d) runs ----
        q_t = inp.tile([P, NCH, D], bf16, tag="qk", name=f"q_t{bh}")
        nc.gpsimd.dma_start(q_t[:], q_d[bh].rearrange("(p c) d -> p c d", p=P))
        k_t = inp.tile([P, NCH, D], bf16, tag="qk", name=f"k_t{bh}")
        nc.gpsimd.dma_start(k_t[:], k_d[bh].rearrange("(p c) d -> p c d", p=P))
        v_aug = vpool.tile([P, NCH, EAUG], bf16, tag="va", name=f"v_aug{bh}")
        nc.gpsimd.memset(v_aug[:, :, D:EAUG], 1.0)
        nc.gpsimd.dma_start(
            v_aug[:, :, 0:D], v_d[bh].rearrange("(p c) d -> p c d", p=P)
        )

        # ---- transpose q, k into [(t,d), u, p]; chunk c = 2u + t ----
        qT = tsb.tile([P, NU, P], bf16, tag="t", name=f"qT{bh}")
        kT = tsb.tile([P, NU, P], bf16, tag="t", name=f"kT{bh}")
        for src, dst in ((q_t, qT), (k_t, kT)):
            for g in range(2):
                pst = ps_small.tile([P, 8, P], bf16, tag="small", name=f"ps_t{bh}g{g}")
                for i in range(8):
                    u = 8 * g + i
                    nc.tensor.transpose(
                        pst[:, i, :], src[:, 2 * u : 2 * u + 2, :], ident_bf
                    )
                nc.vector.tensor_copy(dst[:, 8 * g : 8 * g + 8, :], pst[:])

        # ---- qp^T = gelu(proj @ q^T)  [m, (t, u, p)] ----
        qpT = feat.tile([P, 2, 2, NU, P], bf16, tag="qpT", name=f"qpT{bh}")
        for mc in range(2):
            for b4 in range(4):  # u in [4b4, 4b4+4)
                psq = ps_gen.tile([P, 1024], f32, tag="gen", name=f"ps_qp{bh}_{mc}{b4}")
                for t in range(2):
                    nc.tensor.matmul(
                        psq[:, 512 * t : 512 * (t + 1)],
                        lhsT=projT[64 * t : 64 * t + 64, P * mc : P * (mc + 1)],
                        rhs=qT[64 * t : 64 * t + 64, 4 * b4 : 4 * b4 + 4, :],
                    )
                nc.scalar.activation(
                    qpT[:, mc, :, 4 * b4 : 4 * b4 + 4, :], psq[:], GELU
                )

        # ---- kp = gelu(k @ proj^T) [n, m]; ctx_aug accumulation ----
        # ms=0 ctx accumulation interleaves with kp production; ms=1 runs
        # after, in a different (full-bank) PSUM tile.
        kp = feat.tile([P, NCH, M], bf16, tag="kp", name=f"kp{bh}")
        kp_v = kp.rearrange("p (j t) m -> p t j m", t=2)
        ctxT = small.tile([P, 2, EAUG], bf16, tag="ctxT", name=f"ctxT{bh}")
        ps_ctx0 = ps_acc.tile(
            [P, EAUG], f32, tag="acc", padded_shape=[P, 512], name=f"psc0{bh}"
        )
        for g in range(8):  # chunks c in [4g, 4g+4)
            psk = ps_gen.tile([P, 1024], f32, tag="gen", name=f"ps_kp{bh}_{g}")
            for jl in range(2):
                j = 2 * g + jl
                for t in range(2):
                    nc.tensor.matmul(
                        psk[:, 256 * (2 * t + jl) : 256 * (2 * t + jl + 1)],
                        lhsT=kT[64 * t : 64 * t + 64, j, :],
                        rhs=projT[64 * t : 64 * t + 64, :],
                    )
            nc.scalar.activation(kp_v[:, :, 2 * g : 2 * g + 2, :], psk[:], GELU)
            for cl in range(4):
                c = 4 * g + cl
                nc.tensor.matmul(
                    ps_ctx0[:],
                    lhsT=kp[:, c, 0:P],
                    rhs=v_aug[:, c, :],
                    start=(c == 0),
                    stop=(c == NCH - 1),
                )
        nc.vector.tensor_copy(ctxT[:, 0, :], ps_ctx0[:])
        ps_ctx1 = ps_acc.tile(
            [P, EAUG], f32, tag="acc", padded_shape=[P, 512], name=f"psc1{bh}"
        )
        for c in range(NCH):
            nc.tensor.matmul(
                ps_ctx1[:],
                lhsT=kp[:, c, P : 2 * P],
                rhs=v_aug[:, c, :],
                start=(c == 0),
                stop=(c == NCH - 1),
            )
        nc.vector.tensor_copy(ctxT[:, 1, :], ps_ctx1[:])

        # ---- out[n, e] = qp @ ctx_aug (col 64 = den), normalize ----
        out_stage = outp.tile([P, NCH, D], f32, tag="ost", name=f"out_stage{bh}")
        for og in range(8):  # chunks c in [4og, 4og+4)
            pso = ps_acc.tile(
                [P, 4, EAUG],
                f32,
                tag="acc",
                padded_shape=[P, 4, P],
                name=f"pso{bh}{og}",
            )
            for cl in range(4):
                c = 4 * og + cl
                t, u = c & 1, c >> 1
                for mc in range(2):
                    nc.tensor.matmul(
                        pso[:, cl, :],
                        lhsT=qpT[:, mc, t, u, :],
                        rhs=ctxT[:, mc, :],
                        start=(mc == 0),
                        stop=(mc == 1),
                    )
            rec = small.tile([P, 4], f32, tag="rec", name=f"rec{bh}{og}")
            nc.vector.reciprocal(rec[:], pso[:, :, D])
            nc.vector.tensor_tensor(
                out_stage[:, 4 * og : 4 * og + 4, :],
                pso[:, :, 0:D],
                rec[:, :, None].to_broadcast((P, 4, D)),
                MULT,
            )
        nc.sync.dma_start(
            out_d[bh].rearrange("(p c) d -> p c d", p=P), out_stage[:]
        )

    if repeat > 1:
        loop_cm.__exit__(None, None, None)


def build(bhpc=BHPC, repeat=1):
    from contextlib import ExitStack

    import concourse.mybir as mybir
    import concourse.tile as tile
    from concourse import bacc

    nc = bacc.Bacc("TRN2", target_bir_lowering=False, debug=False)
    f32 = mybir.dt.float32
    q_d = nc.dram_tensor("q", [bhpc, N, D], f32, kind="ExternalInput").ap()
    k_d = nc.dram_tensor("k", [bhpc, N, D], f32, kind="ExternalInput").ap()
    v_d = nc.dram_tensor("v", [bhpc, N, D], f32, kind="ExternalInput").ap()
    proj_d = nc.dram_tensor("proj_mat", [M, D], f32, kind="ExternalInput").ap()
    out_d = nc.dram_tensor("out", [bhpc, N, D], f32, kind="ExternalOutput").ap()

    with tile.TileContext(nc) as tc:
        with ExitStack() as body_ctx:
            _emit_body(body_ctx, tc, out_d, q_d, k_d, v_d, proj_d, bhpc, repeat)
    nc.compile()
    return nc


_built = None


def _get_built():
    global _built
    if _built is None:
        _built = build()
    return _built


def _shard_inputs(q, k, v, proj_mat):
    qf = np.ascontiguousarray(q.reshape(BH, N, D), dtype=np.float32)
    kf = np.ascontiguousarray(k.reshape(BH, N, D), dtype=np.float32)
    vf = np.ascontiguousarray(v.reshape(BH, N, D), dtype=np.float32)
    pf = np.ascontiguousarray(proj_mat, dtype=np.float32)
    in_maps = []
    for c in range(NCORES):
        s = slice(c * BHPC, (c + 1) * BHPC)
        in_maps.append({"q": qf[s], "k": kf[s], "v": vf[s], "proj_mat": pf})
    return in_maps


def run_on_hw(q, k, v, proj_mat, trace=False, **kwargs):
    from concourse.bass_utils import run_bass_kernel_spmd

    nc = _get_built()
    in_maps = _shard_inputs(q, k, v, proj_mat)
    res = run_bass_kernel_spmd(
        nc, in_maps, core_ids=list(range(NCORES)), trace=trace, **kwargs
    )
    out = np.concatenate([r["out"] for r in res.results], axis=0)
    return out.reshape(B, H, N, D).astype(np.float32), res


def kernel(q, k, v, proj_mat):
    out, _ = run_on_hw(q, k, v, proj_mat, trace=False)
    return out


# revision 22
# speedup vs baseline: 1.1741x; 1.1741x over previous
"""Trainium2 Bass kernel for nn_GeneralizedAttention (Performer-style linear
attention with GELU random features).

Math (per (b,h)):
    qp  = gelu(q @ proj^T)            [n, m]
    kp  = gelu(k @ proj^T)            [n, m]
    ksum= kp.sum(n)                   [m]   (v ones column)
    ctx = kp^T @ v                    [m, e]
    den = qp @ ksum                   [n]   (ctx_aug col 64)
    out = (qp @ ctx) / den[:, None]   [n, e]

Sharding: B*H = 64 (b,h) pairs split across 8 cores, 8 pairs each; proj_mat
replicated; no cross-core comms.

Layout: n is chunked as n = p*32 + c (p = SBUF partition, c = chunk id, 32
chunks of 128); all HBM<->SBUF transfers are contiguous in (c, d) per
partition (4-8KB descriptor runs -> full DMA bus efficiency). Chunks pair
(2u, 2u+1) onto partition halves (t*64+d) of qT/kT so projection matmuls
contract d=64 on both row halves. The ctx and out contractions stream the
narrow e+1=65 axis with the feature tiles as stationary weights (half the
PE time of streaming m), and produce `out` directly in [n, e] — no output
transposes. Accumulation groups never interleave within a PSUM bank (a
group start clears has_written for the whole bank), and every PSUM
accumulator tile is padded to a full 2KB bank.
"""

import numpy as np

B, H, N, D, M = 4, 16, 4096, 64, 256
NCORES = 8
BH = B * H
BHPC = BH // NCORES  # 8 (b,h) pairs per core
P = 128
NCH = N // P         # 32 chunks of 128 n
NU = NCH // 2        # 16 chunk pairs
EAUG = D + 1         # 65: e plus the folded ksum/den column


def _emit_body(ctx, tc, out_d, q_d, k_d, v_d, proj_d, bhpc, repeat=1):
    import concourse.bass as bass
    import concourse.mybir as mybir
    from concourse.masks import make_identity

    nc = tc.nc
    f32 = mybir.dt.float32
    bf16 = mybir.dt.bfloat16
    MULT = mybir.AluOpType.mult
    GELU = mybir.ActivationFunctionType.Gelu

    const = ctx.enter_context(tc.tile_pool(name="const", bufs=1))
    inp = ctx.enter_context(tc.tile_pool(name="inp", bufs=6))
    vpool = ctx.enter_context(tc.tile_pool(name="vpool", bufs=3))
    tsb = ctx.enter_context(tc.tile_pool(name="tsb", bufs=6))
    feat = ctx.enter_context(tc.tile_pool(name="feat", bufs=3))
    small = ctx.enter_context(tc.tile_pool(name="small", bufs=3))
    outp = ctx.enter_context(tc.tile_pool(name="outp", bufs=3))
    ps_gen = ctx.enter_context(tc.tile_pool(name="ps_gen", bufs=2, space="PSUM"))
    ps_small = ctx.enter_context(tc.tile_pool(name="ps_small", bufs=2, space="PSUM"))
    ps_acc = ctx.enter_context(tc.tile_pool(name="ps_acc", bufs=2, space="PSUM"))

    ident_bf = const.tile([P, P], bf16, name="ident_bf")
    make_identity(nc, ident_bf)
    ident_f32 = const.tile([P, P], f32, name="ident_f32")
    make_identity(nc, ident_f32)

    # proj^T [d, m] duplicated on both partition halves (rows 0-63 and 64-127)
    proj_nat = const.tile([P, 2, D], f32, name="proj_nat")
    nc.sync.dma_start(proj_nat[:], proj_d.rearrange("(t p) d -> p t d", p=P))
    projT = const.tile([P, M], bf16, name="projT")
    for t in range(2):
        pspt = ps_small.tile([D, P], f32, tag="small", name=f"ps_projT{t}")
        nc.tensor.transpose(pspt[:], proj_nat[:, t, :], ident_f32)
        nc.vector.tensor_copy(projT[0:D, P * t : P * (t + 1)], pspt[:])
        nc.vector.tensor_copy(projT[D:P, P * t : P * (t + 1)], pspt[:])

    unroll = 1
    if repeat > 1:
        # amortize the For_i semaphore-reset barrier over several passes
        for cand in (8, 4, 2):
            if repeat % cand == 0:
                unroll = cand
                break
        loop_cm = tc.For_i(0, repeat // unroll, 1)
        loop_cm.__enter__()

    for bh in range(bhpc * unroll):
        bh = bh % bhpc
        # ---- loads (SWDGE cast f32 -> bf16), contiguous (c,d) runs ----
        q_t = inp.tile([P, NCH, D], bf16, tag="qk", name=f"q_t{bh}")
        nc.gpsimd.dma_start(q_t[:], q_d[bh].rearrange("(p c) d -> p c d", p=P))
        k_t = inp.tile([P, NCH, D], bf16, tag="qk", name=f"k_t{bh}")
        nc.gpsimd.dma_start(k_t[:], k_d[bh].rearrange("(p c) d -> p c d", p=P))
        v_aug = vpool.tile([P, NCH, EAUG], bf16, tag="va", name=f"v_aug{bh}")
        nc.gpsimd.memset(v_aug[:, :, D:EAUG], 1.0)
        nc.gpsimd.dma_start(
            v_aug[:, :, 0:D], v_d[bh].rearrange("(p c) d -> p c d", p=P)
        )

        # ---- transpose q, k into [(t,d), u, p]; chunk c = 2u + t ----
        qT = tsb.tile([P, NU, P], bf16, tag="t", name=f"qT{bh}")
        kT = tsb.tile([P, NU, P], bf16, tag="t", name=f"kT{bh}")
        for src, dst in ((q_t, qT), (k_t, kT)):
            for g in range(2):
                pst = ps_small.tile([P, 8, P], bf16, tag="small", name=f"ps_t{bh}g{g}")
                for i in range(8):
                    u = 8 * g + i
                    nc.tensor.transpose(
                        pst[:, i, :], src[:, 2 * u : 2 * u + 2, :], ident_bf
                    )
                nc.vector.tensor_copy(dst[:, 8 * g : 8 * g + 8, :], pst[:])

        # ---- qp^T = gelu(proj @ q^T)  [m, (t, u, p)] ----
        qpT = feat.tile([P, 2, 2, NU, P], bf16, tag="qpT", name=f"qpT{bh}")
        for mc in range(2):
            for b4 in range(4):  # u in [4b4, 4b4+4)
                psq = ps_gen.tile([P, 1024], f32, tag="gen", name=f"ps_qp{bh}_{mc}{b4}")
                for t in range(2):
                    nc.tensor.matmul(
                        psq[:, 512 * t : 512 * (t + 1)],
                        lhsT=projT[64 * t : 64 * t + 64, P * mc : P * (mc + 1)],
                        rhs=qT[64 * t : 64 * t + 64, 4 * b4 : 4 * b4 + 4, :],
                    )
                nc.scalar.activation(
                    qpT[:, mc, :, 4 * b4 : 4 * b4 + 4, :], psq[:], GELU
                )

        # ---- kp = gelu(k @ proj^T) [n, m]; ctx_aug accumulation ----
        # ms=0 ctx accumulation interleaves with kp production; ms=1 runs
        # after, in a different (full-bank) PSUM tile.
        kp = feat.tile([P, NCH, M], bf16, tag="kp", name=f"kp{bh}")
        kp_v = kp.rearrange("p (j t) m -> p t j m", t=2)
        ctxT = small.tile([P, 2, EAUG], bf16, tag="ctxT", name=f"ctxT{bh}")
        ps_ctx0 = ps_acc.tile(
            [P, EAUG], f32, tag="acc", padded_shape=[P, 512], name=f"psc0{bh}"
        )
        for g in range(8):  # chunks c in [4g, 4g+4)
            psk = ps_gen.tile([P, 1024], f32, tag="gen", name=f"ps_kp{bh}_{g}")
            for jl in range(2):
                j = 2 * g + jl
                for t in range(2):
                    nc.tensor.matmul(
                        psk[:, 256 * (2 * t + jl) : 256 * (2 * t + jl + 1)],
                        lhsT=kT[64 * t : 64 * t + 64, j, :],
                        rhs=projT[64 * t : 64 * t + 64, :],
                    )
            nc.scalar.activation(kp_v[:, :, 2 * g : 2 * g + 2, :], psk[:], GELU)
            for cl in range(4):
                c = 4 * g + cl
                nc.tensor.matmul(
                    ps_ctx0[:],
                    lhsT=kp[:, c, 0:P],
                    rhs=v_aug[:, c, :],
                    start=(c == 0),
                    stop=(c == NCH - 1),
                )
        nc.vector.tensor_copy(ctxT[:, 0, :], ps_ctx0[:])
        ps_ctx1 = ps_acc.tile(
            [P, EAUG], f32, tag="acc", padded_shape=[P, 512], name=f"psc1{bh}"
        )
        for c in range(NCH):
            nc.tensor.matmul(
                ps_ctx1[:],
                lhsT=kp[:, c, P : 2 * P],
                rhs=v_aug[:, c, :],
                start=(c == 0),
                stop=(c == NCH - 1),
            )
        nc.vector.tensor_copy(ctxT[:, 1, :], ps_ctx1[:])

        # ---- out[n, e] = qp @ ctx_aug (col 64 = den), normalize ----
        out_stage = outp.tile([P, NCH, D], f32, tag="ost", name=f"out_stage{bh}")
        for og in range(8):  # chunks c in [4og, 4og+4)
            pso = ps_acc.tile(
                [P, 4, EAUG],
                f32,
                tag="acc",
                padded_shape=[P, 4, P],
                name=f"pso{bh}{og}",
            )
            for cl in range(4):
                c = 4 * og + cl
                t, u = c & 1, c >> 1
                for mc in range(2):
                    nc.tensor.matmul(
                        pso[:, cl, :],
                        lhsT=qpT[:, mc, t, u, :],
                        rhs=ctxT[:, mc, :],
                        start=(mc == 0),
                        stop=(mc == 1),
                    )
            rec = small.tile([P, 4], f32, tag="rec", name=f"rec{bh}{og}")
            nc.vector.reciprocal(rec[:], pso[:, :, D])
            nc.vector.tensor_tensor(
                out_stage[:, 4 * og : 4 * og + 4, :],
                pso[:, :, 0:D],
                rec[:, :, None].to_broadcast((P, 4, D)),
                MULT,
            )
        nc.sync.dma_start(
            out_d[bh].rearrange("(p c) d -> p c d", p=P), out_stage[:]
        )

    if repeat > 1:
        loop_cm.__exit__(None, None, None)


def build(bhpc=BHPC, repeat=1):
    from contextlib import ExitStack

    import concourse.mybir as mybir
    import concourse.tile as tile
    from concourse import bacc

    nc = bacc.Bacc("TRN2", target_bir_lowering=False, debug=False)
    f32 = mybir.dt.float32
    q_d = nc.dram_tensor("q", [bhpc, N, D], f32, kind="ExternalInput").ap()
    k_d = nc.dram_tensor("k", [bhpc, N, D], f32, kind="ExternalInput").ap()
    v_d = nc.dram_tensor("v", [bhpc, N, D], f32, kind="ExternalInput").ap()
    proj_d = nc.dram_tensor("proj_mat", [M, D], f32, kind="ExternalInput").ap()
    out_d = nc.dram_tensor("out", [bhpc, N, D], f32, kind="ExternalOutput").ap()

    with tile.TileContext(nc) as tc:
        with ExitStack() as body_ctx:
            _emit_body(body_ctx, tc, out_d, q_d, k_d, v_d, proj_d, bhpc, repeat)
    nc.compile()
    return nc


_built = None


def _get_built():
    global _built
    if _built is None:
        _built = build()
    return _built


def _shard_inputs(q, k, v, proj_mat):
    qf = np.ascontiguousarray(q.reshape(BH, N, D), dtype=np.float32)
    kf = np.ascontiguousarray(k.reshape(BH, N, D), dtype=np.float32)
    vf = np.ascontiguousarray(v.reshape(BH, N, D), dtype=np.float32)
    pf = np.ascontiguousarray(proj_mat, dtype=np.float32)
    in_maps = []
    for c in range(NCORES):
        s = slice(c * BHPC, (c + 1) * BHPC)
        in_maps.append({"q": qf[s], "k": kf[s], "v": vf[s], "proj_mat": pf})
    return in_maps


def run_on_hw(q, k, v, proj_mat, trace=False, **kwargs):
    from concourse.bass_utils import run_bass_kernel_spmd

    nc = _get_built()
    in_maps = _shard_inputs(q, k, v, proj_mat)
    res = run_bass_kernel_spmd(
        nc, in_maps, core_ids=list(range(NCORES)), trace=trace, **kwargs
    )
    out = np.concatenate([r["out"] for r in res.results], axis=0)
    return out.reshape(B, H, N, D).astype(np.float32), res


def kernel(q, k, v, proj_mat):
    out, _ = run_on_hw(q, k, v, proj_mat, trace=False)
    return out
